# Initial kernel scaffold
#
"""Trainium2 Bass kernel for CropPoolLayer (TF crop_and_resize bilinear + 2x2 maxpool).

Decomposition (host precomputes indices + factored bilinear weights):
  per ROI: crops[(i,j), c] = sum_q W[(i,j), q] * bottom_flat[pt[q], c]
  pooled = 2x2 max over (i,j)

Device (per core, SPMD over 8 cores, 64 ROIs each):
  - dma_gather of each ROI's feature-map bbox points (rows of 512 ch, 2KB each)
  - two 98-column f32r matmuls per ROI (even/odd crop rows) accumulating over
    2 gather columns (256 pts capacity; pts>256 tails go to overflow groups)
  - pool step1 = max(psumA, psumB) [98,512]; step2 = max(v[0:49], v[49:98])
  - M-order m = dj*49 + pi*7 + pj with (i,j) = (2pi+di, 2pj+dj) makes both
    pool steps plain partition-window ops.
"""
import sys

sys.path.insert(0, "/opt/trn_rl_repo")

import numpy as np

POOL = 7
CROP = 14
B, H, W, C = 2, 64, 64, 512
NROI = 512
NCORES = 8
NR = NROI // NCORES           # 64 ROIs (main groups) per core
N_OVERFLOW = 8                # overflow groups per core
T_COLS = 2 * NR + N_OVERFLOW  # 136 gather cols per core
MAIN_PTS = 256
NIDX = T_COLS * 128           # 17408 gather positions per core
COLS_PER_CHUNK = 8


def round_f32r(x):
    """Round-to-nearest-even dropping the low 12 mantissa bits (f32r grid)."""
    b = np.ascontiguousarray(x, dtype=np.float32).view(np.uint32).astype(np.uint64)
    low = b & 0xFFF
    base = b & ~np.uint64(0xFFF)
    up = (low > 0x800) | ((low == 0x800) & (((b >> 12) & 1) == 1))
    out = base + np.where(up, np.uint64(0x1000), np.uint64(0))
    return out.astype(np.uint32).view(np.float32).reshape(np.shape(x))


def _grid_geometry(rois, im_info):
    rois = np.asarray(rois, dtype=np.float32)
    im_h = np.float32(im_info[0])
    im_w = np.float32(im_info[1])
    bids = rois[:, 0].astype(np.int32)
    x1 = rois[:, 1] / im_w
    y1 = rois[:, 2] / im_h
    x2 = rois[:, 3] / im_w
    y2 = rois[:, 4] / im_h
    grid = np.arange(CROP, dtype=np.float32)
    ys = y1[:, None] * np.float32(H - 1) + grid[None, :] * ((y2 - y1) * np.float32(H - 1) / np.float32(CROP - 1))[:, None]
    xs = x1[:, None] * np.float32(W - 1) + grid[None, :] * ((x2 - x1) * np.float32(W - 1) / np.float32(CROP - 1))[:, None]
    vy = (ys >= 0) & (ys <= H - 1)
    vx = (xs >= 0) & (xs <= W - 1)
    y0f = np.floor(ys)
    x0f = np.floor(xs)
    ly = (ys - y0f).astype(np.float32)
    lx = (xs - x0f).astype(np.float32)
    y0 = np.clip(y0f, 0, H - 1).astype(np.int32)
    y1i = np.clip(np.ceil(ys), 0, H - 1).astype(np.int32)
    x0 = np.clip(x0f, 0, W - 1).astype(np.int32)
    x1i = np.clip(np.ceil(xs), 0, W - 1).astype(np.int32)
    return dict(bids=bids, ly=ly, lx=lx, y0=y0, y1i=y1i, x0=x0, x1i=x1i, vy=vy, vx=vx)


# m = dj*64 + pi*7 + pj; columns 49..63 / 113..127 are zero padding so the
# pool step-2 partition windows start at 0 and 64 (hardware alignment rule)
_M_ENTITY = np.full((2, 128, 2), -1, np.int32)
for _di in range(2):
    for _dj in range(2):
        for _pi in range(POOL):
            for _pj in range(POOL):
                _M_ENTITY[_di, _dj * 64 + _pi * 7 + _pj] = (2 * _pi + _di, 2 * _pj + _dj)
_M_VALID = _M_ENTITY[0, :, 0] >= 0  # [128]


def _roi_weights(g, n):
    y0 = g["y0"][n]; y1i = g["y1i"][n]; x0 = g["x0"][n]; x1i = g["x1i"][n]
    ly = g["ly"][n]; lx = g["lx"][n]; vy = g["vy"][n]; vx = g["vx"][n]
    rmin = int(min(y0.min(), y1i.min())); rmax = int(max(y0.max(), y1i.max()))
    cmin = int(min(x0.min(), x1i.min())); cmax = int(max(x0.max(), x1i.max()))
    nrows = rmax - rmin + 1; ncols = cmax - cmin + 1
    Wy = np.zeros((CROP, nrows), np.float32)
    Wx = np.zeros((CROP, ncols), np.float32)
    ii = np.arange(CROP)
    np.add.at(Wy, (ii, y0 - rmin), ((1.0 - ly) * vy).astype(np.float32))
    np.add.at(Wy, (ii, y1i - rmin), (ly * vy).astype(np.float32))
    np.add.at(Wx, (ii, x0 - cmin), ((1.0 - lx) * vx).astype(np.float32))
    np.add.at(Wx, (ii, x1i - cmin), (lx * vx).astype(np.float32))
    Wfull = np.einsum("ir,jx->ijrx", Wy, Wx).reshape(CROP * CROP, nrows * ncols)
    return int(g["bids"][n]), rmin, cmin, nrows, ncols, Wfull


def _assign_rois_to_cores(g):
    pts = np.empty(NROI, np.int64)
    for n in range(NROI):
        y0 = g["y0"][n]; y1i = g["y1i"][n]; x0 = g["x0"][n]; x1i = g["x1i"][n]
        nrows = int(max(y0.max(), y1i.max())) - int(min(y0.min(), y1i.min())) + 1
        ncols = int(max(x0.max(), x1i.max())) - int(min(x0.min(), x1i.min())) + 1
        pts[n] = nrows * ncols
    order = np.argsort(pts, kind="stable")
    cores = [[] for _ in range(NCORES)]
    for rank, n in enumerate(order):
        rnd, pos = divmod(rank, NCORES)
        c = pos if rnd % 2 == 0 else NCORES - 1 - pos
        cores[c].append(int(n))
    for c in range(NCORES):
        big = sum(pts[n] > MAIN_PTS for n in cores[c])
        assert big <= N_OVERFLOW, f"core {c}: {big} ROIs with pts>{MAIN_PTS}"
    return cores


def _build_core_plan(g, roi_ids):
    MW = 128
    gidx = np.full((T_COLS * 128,), -1, np.int16)
    w_main = np.zeros((NR, 128, 4 * MW), np.float32)
    w_ov = np.zeros((N_OVERFLOW, 128, 2 * MW), np.float32)
    for slot, n in enumerate(roi_ids):
        bid, rmin, cmin, nrows, ncols, Wfull = _roi_weights(g, n)
        pts = nrows * ncols
        rr, xx = np.divmod(np.arange(pts), ncols)
        flat_idx = (bid * (H * W) + (rmin + rr) * W + (cmin + xx)).astype(np.int16)
        npt_main = min(pts, MAIN_PTS)
        gidx[slot * 256: slot * 256 + npt_main] = flat_idx[:npt_main]
        for chunk in (0, 1):
            ent = _M_ENTITY[chunk]
            Wsel = np.zeros((MW, pts), np.float32)
            Wsel[_M_VALID] = Wfull[ent[_M_VALID, 0] * CROP + ent[_M_VALID, 1]]
            w_main[slot][: min(128, npt_main), MW * chunk: MW * (chunk + 1)] = \
                Wsel[:, : min(128, npt_main)].T
            if npt_main > 128:
                w_main[slot][: npt_main - 128, MW * (2 + chunk): MW * (3 + chunk)] = \
                    Wsel[:, 128:npt_main].T
            if pts > MAIN_PTS:
                assert slot >= NR - N_OVERFLOW
                w_ov[slot - (NR - N_OVERFLOW)][: pts - MAIN_PTS, MW * chunk: MW * (chunk + 1)] = \
                    Wsel[:, MAIN_PTS:].T
        if pts > MAIN_PTS:
            ov = slot - (NR - N_OVERFLOW)
            gidx[(2 * NR + ov) * 128: (2 * NR + ov) * 128 + pts - MAIN_PTS] = flat_idx[MAIN_PTS:]
    # First uses of each SBUF gather slot must be fully written: pad the first
    # PAD_MAIN main windows and every overflow window with valid dummy idx 0
    # (their lhsT rows are zero). Later windows inherit finite stale data.
    PAD_MAIN = 0
    for s in range(PAD_MAIN):
        w = gidx[s * 256: (s + 1) * 256]
        w[w < 0] = 0
    for og in range(N_OVERFLOW):
        base = (2 * NR + og) * 128
        w = gidx[base: base + 128]
        w[w < 0] = 0
    gcnt = np.zeros((1, NR + N_OVERFLOW), np.uint32)
    for s in range(NR):
        gcnt[0, s] = np.count_nonzero(gidx[s * 256: (s + 1) * 256] >= 0)
    for og in range(N_OVERFLOW):
        base = (2 * NR + og) * 128
        gcnt[0, NR + og] = np.count_nonzero(gidx[base: base + 128] >= 0)
    it = np.full((16, T_COLS * 8), -1, np.int16)
    gpos = np.arange(T_COLS * 128)
    it[gpos % 16, gpos // 16] = gidx
    return dict(gidx=np.tile(it, (8, 1)), w=round_f32r(w_main), wov=round_f32r(w_ov), gcnt=gcnt)


_NC_CACHE = {}


def _build_bass(repeat=1):
    if repeat in _NC_CACHE:
        return _NC_CACHE[repeat]
    import concourse.bacc as bacc
    import concourse.mybir as mybir
    from concourse.tile import TileContext

    f32 = mybir.dt.float32
    f32r = mybir.dt.float32r
    nc = bacc.Bacc("TRN2", target_bir_lowering=False, debug=False, num_devices=NCORES)
    bottom = nc.dram_tensor("bottom", [B * H * W, C], f32r, kind="ExternalInput")
    gidx = nc.dram_tensor("gidx", [128, T_COLS * 8], mybir.dt.int16, kind="ExternalInput")
    wmain = nc.dram_tensor("w", [NR, 128, 4 * 128], f32r, kind="ExternalInput")
    wov = nc.dram_tensor("wov", [N_OVERFLOW, 128, 2 * 128], f32r, kind="ExternalInput")
    gcnt = nc.dram_tensor("gcnt", [1, NR + N_OVERFLOW], mybir.dt.uint32, kind="ExternalInput")
    out = nc.dram_tensor("out", [NR, 49, C], f32, kind="ExternalOutput")

    mmax = mybir.AluOpType.max

    with TileContext(nc) as tc:
        with (
            tc.tile_pool(name="idxp", bufs=1) as idxp,
            tc.tile_pool(name="gp", bufs=3) as gp,
            tc.tile_pool(name="wp", bufs=4) as wp,
            tc.tile_pool(name="ovp", bufs=1) as ovp,
            tc.tile_pool(name="vp", bufs=2) as vp,
            tc.tile_pool(name="pwp", bufs=2) as pwp,
            tc.tile_pool(name="tmpp", bufs=4) as tmpp,
            tc.tile_pool(name="psp", bufs=4, space="PSUM") as psp,
        ):
            it = idxp.tile([128, T_COLS * 8], mybir.dt.int16, tag="idx")
            nc.sync.dma_start(out=it[:], in_=gidx[:])
            ct = idxp.tile([1, NR + N_OVERFLOW], mybir.dt.uint32, tag="cnt")
            nc.sync.dma_start(out=ct[:], in_=gcnt[:])

            for _rep in range(repeat):
                # ---- overflow groups first (their results persist in SBUF) ----
                creg = nc.gpsimd.alloc_register("gcnt_reg")
                if _rep == 0:
                    # zero-fill every gather slot once: skipped gather tails must
                    # never expose NaN to the PE (NaN * 0 = NaN)
                    zsl = []
                    for _z in range(3):
                        zg = gp.tile([128, 2, C], f32r, tag="gt")
                        nc.gpsimd.memset(zg[:].bitcast(mybir.dt.float32), 0.0)
                        zo = gp.tile([128, 1, C], f32r, tag="ovgt")
                        nc.gpsimd.memset(zo[:].bitcast(mybir.dt.float32), 0.0)
                ov_sb = []
                for og in range(N_OVERFLOW):
                    ovgt = gp.tile([128, 1, C], f32r, tag="ovgt")
                    nc.gpsimd.reg_load(creg, ct[0:1, NR + og: NR + og + 1])
                    nc.gpsimd.dma_gather(
                        out_ap=ovgt[:], in_ap=bottom[:],
                        idxs_ap=it[:, (2 * NR + og) * 8: (2 * NR + og + 1) * 8],
                        num_idxs=128, num_idxs_reg=creg,
                        elem_size=C,
                    )
                    wt = wp.tile([128, 2 * 128], f32r, tag="wov")
                    nc.sync.dma_start(out=wt[:], in_=wov[og])
                    rhs = ovgt[:, 0, :]
                    sbA = ovp.tile([128, C], f32, tag=f"ovA{og}")
                    sbB = ovp.tile([128, C], f32, tag=f"ovB{og}")
                    psA = psp.tile([128, C], f32, tag="psA")
                    psB = psp.tile([128, C], f32, tag="psB")
                    nc.tensor.matmul(out=psA[:], lhsT=wt[:, 0:128], rhs=rhs, start=True, stop=True)
                    nc.tensor.matmul(out=psB[:], lhsT=wt[:, 128:256], rhs=rhs, start=True, stop=True)
                    nc.scalar.copy(out=sbA[:], in_=psA[:])
                    nc.scalar.copy(out=sbB[:], in_=psB[:])
                    ov_sb.append((sbA, sbB))

                # ---- main groups: 16 chunks x 4 groups ----
                for ci in range(NR // 4):
                    vw = vp.tile([128, 4, C], f32, tag="vw")
                    for gi in range(4):
                        slot = 4 * ci + gi
                        gt = gp.tile([128, 2, C], f32r, tag="gt")
                        nc.gpsimd.reg_load(creg, ct[0:1, slot: slot + 1])
                        nc.gpsimd.dma_gather(
                            out_ap=gt[:], in_ap=bottom[:],
                            idxs_ap=it[:, slot * 16: (slot + 1) * 16],
                            num_idxs=256, num_idxs_reg=creg,
                            elem_size=C,
                        )
                        wt = wp.tile([128, 4 * 128], f32r, tag="w")
                        nc.sync.dma_start(out=wt[:], in_=wmain[slot])
                        rhs0 = gt[:, 0, :]
                        rhs1 = gt[:, 1, :]
                        psA = psp.tile([128, C], f32, tag="psA")
                        psB = psp.tile([128, C], f32, tag="psB")
                        nc.tensor.matmul(out=psA[:], lhsT=wt[:, 0:128], rhs=rhs0, start=True, stop=False)
                        nc.tensor.matmul(out=psA[:], lhsT=wt[:, 256:384], rhs=rhs1, start=False, stop=True)
                        nc.tensor.matmul(out=psB[:], lhsT=wt[:, 128:256], rhs=rhs0, start=True, stop=False)
                        nc.tensor.matmul(out=psB[:], lhsT=wt[:, 384:512], rhs=rhs1, start=False, stop=True)
                        if slot >= NR - N_OVERFLOW:
                            sbA, sbB = ov_sb[slot - (NR - N_OVERFLOW)]
                            tmpA = tmpp.tile([128, C], f32, tag="tmpA")
                            tmpB = tmpp.tile([128, C], f32, tag="tmpB")
                            nc.vector.tensor_add(out=tmpA[:], in0=psA[:], in1=sbA[:])
                            nc.vector.tensor_add(out=tmpB[:], in0=psB[:], in1=sbB[:])
                            nc.vector.tensor_tensor(out=vw[:, gi, :], in0=tmpA[:], in1=tmpB[:], op=mmax)
                        else:
                            # tensor_tensor allows only one PSUM operand:
                            # evacuate A on ACT, max against B on DVE
                            sbA = tmpp.tile([128, C], f32, tag="tmpA")
                            nc.scalar.copy(out=sbA[:], in_=psA[:])
                            nc.vector.tensor_tensor(out=vw[:, gi, :], in0=psB[:], in1=sbA[:], op=mmax)
                    # dj-fold: SB+SB tensor ops need equal base partitions, so
                    # DMA the hi half (base 64) into a base-0 tile first
                    vhi = pwp.tile([64, 4, C], f32, tag="vhi")
                    nc.sync.dma_start(out=vhi[:], in_=vw[64:128, :, :])
                    pw = pwp.tile([49, 4, C], f32, tag="pw")
                    nc.vector.tensor_tensor(out=pw[:], in0=vw[0:49, :, :], in1=vhi[0:49, :, :], op=mmax)
                    nc.scalar.dma_start(
                        out=out[4 * ci: 4 * ci + 4].rearrange("g p c -> p g c"),
                        in_=pw[:],
                    )
    nc.compile()
    _NC_CACHE[repeat] = nc
    return nc


def _prepare_inputs(bottom, rois, im_info):
    g = _grid_geometry(rois, im_info)
    cores = _assign_rois_to_cores(g)
    flat = round_f32r(np.ascontiguousarray(np.asarray(bottom, np.float32).reshape(B * H * W, C)))
    in_maps = []
    for c in range(NCORES):
        p = _build_core_plan(g, cores[c])
        in_maps.append({"bottom": flat, "gidx": p["gidx"], "w": p["w"], "wov": p["wov"], "gcnt": p["gcnt"]})
    return cores, in_maps


def kernel(bottom, rois, im_info):
    from concourse.bass_utils import run_bass_kernel_spmd

    nc = _build_bass()
    cores, in_maps = _prepare_inputs(bottom, rois, im_info)
    res = run_bass_kernel_spmd(nc, in_maps, core_ids=list(range(NCORES)))
    out = np.empty((NROI, POOL, POOL, C), np.float32)
    for c in range(NCORES):
        out[np.array(cores[c])] = res.results[c]["out"].reshape(NR, POOL, POOL, C)
    return out



# revision 8
# speedup vs baseline: 2.4366x; 2.4366x over previous
"""Trainium2 Bass kernel for CropPoolLayer (TF crop_and_resize bilinear + 2x2 maxpool).

Decomposition (host precomputes indices + dense per-ROI bilinear weights):
  per ROI: crops[(i,j), c] = sum_q W[q, (i,j)] * bottom_flat[pt[q], c]
  pooled  = 2x2 max over (i,j)

Device (per core, SPMD over 8 cores, 64 ROIs each, all wire data bf16):
  - per 8-ROI chunk: one batched dma_gather of all chunk points (rows of
    512 ch, 1KB bf16 each) into [128, cols, 512]; ROIs packed at
    64-partition granularity (column = 128 points).
  - transposed matmuls: lhsT = gathered data [pts, 128 c-chunk], rhs =
    weights [pts, 98] -> psum [128(c), 98] so channel is the psum
    partition dim and the (i,j) samples live on the FREE axis.
    Two psum tiles per ROI: psE (even i), psO (odd i), each [128, 4(cc),
    2(dj), 49(pi*7+pj)].
  - pool: ACT evacuates psE -> sbE; DVE max(psO, sbE) folds di; Pool
    engine folds dj on the free axis -> pooled [128, 4, 49] bf16.
  - output written as [pair, 128(c%128), 2(roi), 4(c//128), 49] bf16,
    un-transposed on host.
"""
import sys

sys.path.insert(0, "/opt/trn_rl_repo")

import numpy as np
import ml_dtypes

POOL = 7
CROP = 14
B, H, W, C = 2, 64, 64, 512
NROI = 512
NCORES = 8
NR = NROI // NCORES            # 64 ROIs per core
SLOTS_PER_CHUNK = 8
NCHUNK = NR // SLOTS_PER_CHUNK  # 8 gather chunks per core

BF16 = ml_dtypes.bfloat16

# m-order permutation: (e, dj, pi*7+pj) -> row (2*pi+e)*14 + (2*pj+dj) of Wfull
_PERM = np.empty((2, 2, 49), np.int64)
for _e in range(2):
    for _dj in range(2):
        for _pi in range(POOL):
            for _pj in range(POOL):
                _PERM[_e, _dj, _pi * 7 + _pj] = (2 * _pi + _e) * 14 + (2 * _pj + _dj)
_PERM_FLAT = _PERM.reshape(-1)


def _grid_geometry(rois, im_info):
    rois = np.asarray(rois, dtype=np.float32)
    im_h = np.float32(im_info[0])
    im_w = np.float32(im_info[1])
    bids = rois[:, 0].astype(np.int32)
    x1 = rois[:, 1] / im_w
    y1 = rois[:, 2] / im_h
    x2 = rois[:, 3] / im_w
    y2 = rois[:, 4] / im_h
    grid = np.arange(CROP, dtype=np.float32)
    ys = y1[:, None] * np.float32(H - 1) + grid[None, :] * ((y2 - y1) * np.float32(H - 1) / np.float32(CROP - 1))[:, None]
    xs = x1[:, None] * np.float32(W - 1) + grid[None, :] * ((x2 - x1) * np.float32(W - 1) / np.float32(CROP - 1))[:, None]
    vy = (ys >= 0) & (ys <= H - 1)
    vx = (xs >= 0) & (xs <= W - 1)
    y0f = np.floor(ys)
    x0f = np.floor(xs)
    ly = (ys - y0f).astype(np.float32)
    lx = (xs - x0f).astype(np.float32)
    y0 = np.clip(y0f, 0, H - 1).astype(np.int32)
    y1i = np.clip(np.ceil(ys), 0, H - 1).astype(np.int32)
    x0 = np.clip(x0f, 0, W - 1).astype(np.int32)
    x1i = np.clip(np.ceil(xs), 0, W - 1).astype(np.int32)
    return dict(bids=bids, ly=ly, lx=lx, y0=y0, y1i=y1i, x0=x0, x1i=x1i, vy=vy, vx=vx)


def _roi_weights(g, n):
    y0 = g["y0"][n]; y1i = g["y1i"][n]; x0 = g["x0"][n]; x1i = g["x1i"][n]
    ly = g["ly"][n]; lx = g["lx"][n]; vy = g["vy"][n]; vx = g["vx"][n]
    rmin = int(min(y0.min(), y1i.min())); rmax = int(max(y0.max(), y1i.max()))
    cmin = int(min(x0.min(), x1i.min())); cmax = int(max(x0.max(), x1i.max()))
    nrows = rmax - rmin + 1; ncols = cmax - cmin + 1
    Wy = np.zeros((CROP, nrows), np.float32)
    Wx = np.zeros((CROP, ncols), np.float32)
    ii = np.arange(CROP)
    np.add.at(Wy, (ii, y0 - rmin), ((1.0 - ly) * vy).astype(np.float32))
    np.add.at(Wy, (ii, y1i - rmin), (ly * vy).astype(np.float32))
    np.add.at(Wx, (ii, x0 - cmin), ((1.0 - lx) * vx).astype(np.float32))
    np.add.at(Wx, (ii, x1i - cmin), (lx * vx).astype(np.float32))
    Wfull = np.einsum("ir,jx->ijrx", Wy, Wx).reshape(CROP * CROP, nrows * ncols)
    return int(g["bids"][n]), rmin, cmin, nrows, ncols, Wfull


def _roi_pts(g):
    pts = np.empty(NROI, np.int64)
    for n in range(NROI):
        y0 = g["y0"][n]; y1i = g["y1i"][n]; x0 = g["x0"][n]; x1i = g["x1i"][n]
        nrows = int(max(y0.max(), y1i.max())) - int(min(y0.min(), y1i.min())) + 1
        ncols = int(max(x0.max(), x1i.max())) - int(min(x0.min(), x1i.min())) + 1
        pts[n] = nrows * ncols
    return pts


def _assign_rois_to_cores(pts):
    """Sort ascending by pts, snake-distribute; per-core slot lists end up
    ascending so cross-core slot maxima stay tight."""
    order = np.argsort(pts, kind="stable")
    cores = [[] for _ in range(NCORES)]
    for rank, n in enumerate(order):
        rnd, pos = divmod(rank, NCORES)
        c = pos if rnd % 2 == 0 else NCORES - 1 - pos
        cores[c].append(int(n))
    return cores


def _make_plan(pts, cores):
    """Shared (cross-core) gather/compute layout.

    Every slot k gets span64[k] = roundup64(max_core pts) point positions,
    packed sequentially; each 8-slot chunk starts on a 128 (column)
    boundary. Returns per-slot q-offsets, per-chunk column ranges and
    per-slot matmul runs (column, partition base, length)."""
    span = [0] * NR
    for k in range(NR):
        m = max(pts[cores[c][k]] for c in range(NCORES))
        span[k] = ((int(m) + 63) // 64) * 64
    slot_q0 = [0] * NR
    chunk_col0 = [0] * (NCHUNK + 1)
    q = 0
    for t in range(NCHUNK):
        assert q % 128 == 0
        chunk_col0[t] = q // 128
        for kk in range(SLOTS_PER_CHUNK):
            k = t * SLOTS_PER_CHUNK + kk
            # HW: psum accumulation groups require base-0 matmul operands, so
            # multi-run (span>64) slots must start on a column boundary; only
            # span-64 slots may sit at offset 64 (their single matmul at base
            # 64 is fine).
            if span[k] > 64 and q % 128 != 0:
                q += 64
            slot_q0[k] = q
            q += span[k]
        q = ((q + 127) // 128) * 128
    chunk_col0[NCHUNK] = q // 128
    totq = q
    runs = []
    for k in range(NR):
        rl = []
        s = slot_q0[k] - chunk_col0[k // SLOTS_PER_CHUNK] * 128
        rem = span[k]
        while rem > 0:
            p0 = s % 128
            g = s // 128
            plen = min(rem, 128 - p0)
            rl.append((g, p0, plen))
            s += plen
            rem -= plen
        runs.append(rl)
    return dict(span=span, slot_q0=slot_q0, chunk_col0=chunk_col0, totq=totq,
                runs=tuple(tuple(r) for r in runs))


_NC_CACHE = {}
_DBG_NCHUNK = NCHUNK  # debug knob: build only the first N chunks


def _build_bass(plan):
    key = (plan["totq"], tuple(plan["chunk_col0"]), plan["runs"], _DBG_NCHUNK)
    if key in _NC_CACHE:
        return _NC_CACHE[key]
    import concourse.bacc as bacc
    import concourse.mybir as mybir
    from concourse.tile import TileContext

    f32 = mybir.dt.float32
    bf16 = mybir.dt.bfloat16
    mmax = mybir.AluOpType.max

    totcols = plan["chunk_col0"][NCHUNK]
    ccol0 = plan["chunk_col0"]
    runs = plan["runs"]

    nc = bacc.Bacc("TRN2", target_bir_lowering=False, debug=False, num_devices=NCORES)
    bottom = nc.dram_tensor("bottom", [B * H * W, C], bf16, kind="ExternalInput")
    gidx = nc.dram_tensor("gidx", [128, plan["totq"] // 16], mybir.dt.int16, kind="ExternalInput")
    wts = nc.dram_tensor("w", [128, totcols, 2, 2, 49], bf16, kind="ExternalInput")
    out = nc.dram_tensor("out", [NR // 2, 128, 2, 4, 49], bf16, kind="ExternalOutput")

    with TileContext(nc) as tc:
        with (
            tc.tile_pool(name="idxp", bufs=3) as idxp,
            tc.tile_pool(name="gp", bufs=3) as gp,
            tc.tile_pool(name="wp", bufs=3) as wp,
            tc.tile_pool(name="ep", bufs=3) as ep,
            tc.tile_pool(name="vp", bufs=3) as vp,
            tc.tile_pool(name="pp", bufs=2) as pp,
            tc.tile_pool(name="psp", bufs=4, space="PSUM") as psp,
        ):
            gts = {}
            wtt = {}

            def issue_loads(t):
                cols = ccol0[t + 1] - ccol0[t]
                it = idxp.tile([128, cols * 8], mybir.dt.int16, tag="idx")
                nc.sync.dma_start(out=it[:], in_=gidx[:, ccol0[t] * 8: ccol0[t + 1] * 8])
                gt = gp.tile([128, cols, C], bf16, tag="gt")
                # firmware caps a single gather at 1024 indices (8 columns)
                for j0 in range(0, cols, 8):
                    gcols = min(8, cols - j0)
                    nidx = gcols * 128
                    nc.gpsimd.dma_gather(
                        out_ap=gt[:, j0:j0 + gcols], in_ap=bottom[:],
                        idxs_ap=it[:, j0 * 8: (j0 + gcols) * 8],
                        num_idxs=nidx, num_idxs_reg=nidx,
                        elem_size=C,
                    )
                wt = wp.tile([128, cols, 2, 2, 49], bf16, tag="wt")
                nc.sync.dma_start(out=wt[:], in_=wts[:, ccol0[t]: ccol0[t + 1]])
                gts[t] = gt
                wtt[t] = wt

            issue_loads(0)
            if _DBG_NCHUNK > 1:
                issue_loads(1)
            pair = None
            for t in range(_DBG_NCHUNK):
                if t + 2 < _DBG_NCHUNK:
                    issue_loads(t + 2)
                gt = gts.pop(t)
                wt = wtt.pop(t)
                for kk in range(SLOTS_PER_CHUNK):
                    k = t * SLOTS_PER_CHUNK + kk
                    rl = runs[k]
                    nrun = len(rl)
                    psE = psp.tile([128, 4, 2, 49], f32, tag="psE")
                    psO = psp.tile([128, 4, 2, 49], f32, tag="psO")
                    for cc in range(4):
                        for ri, (g, p0, plen) in enumerate(rl):
                            lhs = gt[p0:p0 + plen, g, cc * 128:(cc + 1) * 128]
                            st = ri == 0
                            sp = ri == nrun - 1
                            nc.tensor.matmul(out=psE[:, cc], lhsT=lhs,
                                             rhs=wt[p0:p0 + plen, g, 0],
                                             start=st, stop=sp)
                            nc.tensor.matmul(out=psO[:, cc], lhsT=lhs,
                                             rhs=wt[p0:p0 + plen, g, 1],
                                             start=st, stop=sp)
                    sbE = ep.tile([128, 4, 2, 49], f32, tag="sbE")
                    nc.scalar.copy(out=sbE[:], in_=psE[:])
                    v = vp.tile([128, 4, 2, 49], bf16, tag="v")
                    nc.vector.tensor_tensor(out=v[:], in0=psO[:], in1=sbE[:], op=mmax)
                    if k % 2 == 0:
                        pair = pp.tile([128, 2, 4, 49], bf16, tag="pair")
                    nc.vector.tensor_tensor(out=pair[:, k % 2], in0=v[:, :, 0],
                                            in1=v[:, :, 1], op=mmax)
                    if k % 2 == 1:
                        nc.scalar.dma_start(out=out[k // 2], in_=pair[:])
    nc.compile()
    _NC_CACHE[key] = nc
    return nc


def _build_core_inputs(g, roi_ids, plan, flat_bf16):
    totq = plan["totq"]
    totcols = plan["chunk_col0"][NCHUNK]
    gq = np.zeros(totq, np.int16)
    wt = np.zeros((128, totcols, 196), np.float32)
    for k, n in enumerate(roi_ids):
        bid, rmin, cmin, nrows, ncols, Wfull = _roi_weights(g, n)
        pts = nrows * ncols
        rr, xx = np.divmod(np.arange(pts), ncols)
        flat_idx = (bid * (H * W) + (rmin + rr) * W + (cmin + xx)).astype(np.int16)
        qs = plan["slot_q0"][k] + np.arange(pts)
        gq[qs] = flat_idx
        Wsel = Wfull[_PERM_FLAT]                     # [196 (e,dj,pp), pts]
        wt[qs % 128, qs // 128, :] = Wsel.T
    it = np.empty((16, totq // 16), np.int16)
    qpos = np.arange(totq)
    it[qpos % 16, qpos // 16] = gq
    return {
        "bottom": flat_bf16,
        "gidx": np.tile(it, (8, 1)),
        "w": wt.reshape(128, totcols, 2, 2, 49).astype(BF16),
    }


def _prepare(bottom, rois, im_info):
    g = _grid_geometry(rois, im_info)
    pts = _roi_pts(g)
    cores = _assign_rois_to_cores(pts)
    plan = _make_plan(pts, cores)
    flat = np.ascontiguousarray(
        np.asarray(bottom, np.float32).reshape(B * H * W, C)).astype(BF16)
    in_maps = [_build_core_inputs(g, cores[c], plan, flat) for c in range(NCORES)]
    return cores, plan, in_maps


def kernel(bottom, rois, im_info):
    from concourse.bass_utils import run_bass_kernel_spmd

    cores, plan, in_maps = _prepare(bottom, rois, im_info)
    nc = _build_bass(plan)
    res = run_bass_kernel_spmd(nc, in_maps, core_ids=list(range(NCORES)))
    out = np.empty((NROI, POOL, POOL, C), np.float32)
    for c in range(NCORES):
        arr = np.asarray(res.results[c]["out"], dtype=np.float32)
        # [pair, p, r2, cc, pp] -> [pair, r2, pp, cc, p] -> [NR, 49, 512]
        vals = arr.transpose(0, 2, 4, 3, 1).reshape(NR, POOL, POOL, C)
        out[np.array(cores[c])] = vals
    return out


# revision 12
# speedup vs baseline: 2.5197x; 1.0341x over previous
"""Trainium2 Bass kernel for CropPoolLayer (TF crop_and_resize bilinear + 2x2 maxpool).

Decomposition (host precomputes indices + dense per-ROI bilinear weights):
  per ROI: crops[(i,j), c] = sum_q W[q, (i,j)] * bottom_flat[pt[q], c]
  pooled  = 2x2 max over (i,j)

Device (per core, SPMD over 8 cores, 64 ROIs each, all wire data bf16):
  - per 8-ROI chunk: one batched dma_gather of all chunk points (rows of
    512 ch, 1KB bf16 each) into [128, cols, 512]; ROIs packed at
    64-partition granularity (column = 128 points).
  - transposed matmuls: lhsT = gathered data [pts, 128 c-chunk], rhs =
    weights [pts, 98] -> psum [128(c), 98] so channel is the psum
    partition dim and the (i,j) samples live on the FREE axis.
    Two psum tiles per ROI: psE (even i), psO (odd i), each [128, 4(cc),
    2(dj), 49(pi*7+pj)].
  - pool: ACT evacuates psE -> sbE; DVE max(psO, sbE) folds di; Pool
    engine folds dj on the free axis -> pooled [128, 4, 49] bf16.
  - output written as [pair, 128(c%128), 2(roi), 4(c//128), 49] bf16,
    un-transposed on host.
"""
import sys

sys.path.insert(0, "/opt/trn_rl_repo")

import numpy as np
import ml_dtypes

POOL = 7
CROP = 14
B, H, W, C = 2, 64, 64, 512
NROI = 512
NCORES = 8
NR = NROI // NCORES            # 64 ROIs per core
SLOTS_PER_CHUNK = 8
NCHUNK = NR // SLOTS_PER_CHUNK  # 8 gather chunks per core

BF16 = ml_dtypes.bfloat16

# m-order permutation: (e, dj, pi*7+pj) -> row (2*pi+e)*14 + (2*pj+dj) of Wfull
_PERM = np.empty((2, 2, 49), np.int64)
for _e in range(2):
    for _dj in range(2):
        for _pi in range(POOL):
            for _pj in range(POOL):
                _PERM[_e, _dj, _pi * 7 + _pj] = (2 * _pi + _e) * 14 + (2 * _pj + _dj)
_PERM_FLAT = _PERM.reshape(-1)


def _grid_geometry(rois, im_info):
    rois = np.asarray(rois, dtype=np.float32)
    im_h = np.float32(im_info[0])
    im_w = np.float32(im_info[1])
    bids = rois[:, 0].astype(np.int32)
    x1 = rois[:, 1] / im_w
    y1 = rois[:, 2] / im_h
    x2 = rois[:, 3] / im_w
    y2 = rois[:, 4] / im_h
    grid = np.arange(CROP, dtype=np.float32)
    ys = y1[:, None] * np.float32(H - 1) + grid[None, :] * ((y2 - y1) * np.float32(H - 1) / np.float32(CROP - 1))[:, None]
    xs = x1[:, None] * np.float32(W - 1) + grid[None, :] * ((x2 - x1) * np.float32(W - 1) / np.float32(CROP - 1))[:, None]
    vy = (ys >= 0) & (ys <= H - 1)
    vx = (xs >= 0) & (xs <= W - 1)
    y0f = np.floor(ys)
    x0f = np.floor(xs)
    ly = (ys - y0f).astype(np.float32)
    lx = (xs - x0f).astype(np.float32)
    y0 = np.clip(y0f, 0, H - 1).astype(np.int32)
    y1i = np.clip(np.ceil(ys), 0, H - 1).astype(np.int32)
    x0 = np.clip(x0f, 0, W - 1).astype(np.int32)
    x1i = np.clip(np.ceil(xs), 0, W - 1).astype(np.int32)
    return dict(bids=bids, ly=ly, lx=lx, y0=y0, y1i=y1i, x0=x0, x1i=x1i, vy=vy, vx=vx)


def _roi_weights(g, n):
    y0 = g["y0"][n]; y1i = g["y1i"][n]; x0 = g["x0"][n]; x1i = g["x1i"][n]
    ly = g["ly"][n]; lx = g["lx"][n]; vy = g["vy"][n]; vx = g["vx"][n]
    rmin = int(min(y0.min(), y1i.min())); rmax = int(max(y0.max(), y1i.max()))
    cmin = int(min(x0.min(), x1i.min())); cmax = int(max(x0.max(), x1i.max()))
    nrows = rmax - rmin + 1; ncols = cmax - cmin + 1
    Wy = np.zeros((CROP, nrows), np.float32)
    Wx = np.zeros((CROP, ncols), np.float32)
    ii = np.arange(CROP)
    np.add.at(Wy, (ii, y0 - rmin), ((1.0 - ly) * vy).astype(np.float32))
    np.add.at(Wy, (ii, y1i - rmin), (ly * vy).astype(np.float32))
    np.add.at(Wx, (ii, x0 - cmin), ((1.0 - lx) * vx).astype(np.float32))
    np.add.at(Wx, (ii, x1i - cmin), (lx * vx).astype(np.float32))
    Wfull = np.einsum("ir,jx->ijrx", Wy, Wx).reshape(CROP * CROP, nrows * ncols)
    return int(g["bids"][n]), rmin, cmin, nrows, ncols, Wfull


def _roi_pts(g):
    pts = np.empty(NROI, np.int64)
    for n in range(NROI):
        y0 = g["y0"][n]; y1i = g["y1i"][n]; x0 = g["x0"][n]; x1i = g["x1i"][n]
        nrows = int(max(y0.max(), y1i.max())) - int(min(y0.min(), y1i.min())) + 1
        ncols = int(max(x0.max(), x1i.max())) - int(min(x0.min(), x1i.min())) + 1
        pts[n] = nrows * ncols
    return pts


def _assign_rois_to_cores(pts):
    """Sort ascending by pts, snake-distribute; per-core slot lists end up
    ascending so cross-core slot maxima stay tight."""
    order = np.argsort(pts, kind="stable")
    cores = [[] for _ in range(NCORES)]
    for rank, n in enumerate(order):
        rnd, pos = divmod(rank, NCORES)
        c = pos if rnd % 2 == 0 else NCORES - 1 - pos
        cores[c].append(int(n))
    return cores


def _make_plan(pts, cores):
    """Shared (cross-core) gather/compute layout.

    Every slot k gets span64[k] = roundup64(max_core pts) point positions,
    packed sequentially; each 8-slot chunk starts on a 128 (column)
    boundary. Returns per-slot q-offsets, per-chunk column ranges and
    per-slot matmul runs (column, partition base, length)."""
    span = [0] * NR
    for k in range(NR):
        m = max(pts[cores[c][k]] for c in range(NCORES))
        span[k] = ((int(m) + 63) // 64) * 64
    # stride-interleave sorted slots into chunks so each chunk mixes small
    # and large ROIs (balances per-chunk gather DMA against compute)
    chunk_slots = [[t + NCHUNK * j for j in range(SLOTS_PER_CHUNK)]
                   for t in range(NCHUNK)]
    emit = [k for cs in chunk_slots for k in cs]
    chunk_of = {}
    for t, cs in enumerate(chunk_slots):
        for k in cs:
            chunk_of[k] = t
    slot_q0 = [0] * NR
    chunk_col0 = [0] * (NCHUNK + 1)
    q = 0
    for t in range(NCHUNK):
        assert q % 128 == 0
        chunk_col0[t] = q // 128
        for k in chunk_slots[t]:
            # HW: psum accumulation groups require base-0 matmul operands, so
            # multi-run (span>64) slots must start on a column boundary; only
            # span-64 slots may sit at offset 64 (their single matmul at base
            # 64 is fine).
            if span[k] > 64 and q % 128 != 0:
                q += 64
            slot_q0[k] = q
            q += span[k]
        q = ((q + 127) // 128) * 128
    chunk_col0[NCHUNK] = q // 128
    totq = q
    runs = []
    for k in range(NR):
        rl = []
        s = slot_q0[k] - chunk_col0[chunk_of[k]] * 128
        rem = span[k]
        while rem > 0:
            p0 = s % 128
            g = s // 128
            plen = min(rem, 128 - p0)
            rl.append((g, p0, plen))
            s += plen
            rem -= plen
        runs.append(rl)
    return dict(span=span, slot_q0=slot_q0, chunk_col0=chunk_col0, totq=totq,
                runs=tuple(tuple(r) for r in runs), emit=tuple(emit),
                chunk_slots=tuple(tuple(cs) for cs in chunk_slots))


_NC_CACHE = {}
_DBG_NCHUNK = NCHUNK  # debug knob: build only the first N chunks


def _build_bass(plan):
    key = (plan["totq"], tuple(plan["chunk_col0"]), plan["runs"], _DBG_NCHUNK)
    if key in _NC_CACHE:
        return _NC_CACHE[key]
    import concourse.bacc as bacc
    import concourse.mybir as mybir
    from concourse.tile import TileContext

    f32 = mybir.dt.float32
    bf16 = mybir.dt.bfloat16
    mmax = mybir.AluOpType.max

    totcols = plan["chunk_col0"][NCHUNK]
    ccol0 = plan["chunk_col0"]
    runs = plan["runs"]

    nc = bacc.Bacc("TRN2", target_bir_lowering=False, debug=False, num_devices=NCORES)
    bottom = nc.dram_tensor("bottom", [B * H * W, C], bf16, kind="ExternalInput")
    gidx = nc.dram_tensor("gidx", [128, plan["totq"] // 16], mybir.dt.int16, kind="ExternalInput")
    wts = nc.dram_tensor("w", [128, totcols, 2, 2, 49], bf16, kind="ExternalInput")
    out = nc.dram_tensor("out", [NR // 2, 128, 2, 4, 49], bf16, kind="ExternalOutput")

    with TileContext(nc) as tc:
        with (
            tc.tile_pool(name="idxp", bufs=3) as idxp,
            tc.tile_pool(name="gp", bufs=3) as gp,
            tc.tile_pool(name="wp", bufs=3) as wp,
            tc.tile_pool(name="ep", bufs=3) as ep,
            tc.tile_pool(name="vp", bufs=3) as vp,
            tc.tile_pool(name="pp", bufs=2) as pp,
            tc.tile_pool(name="psp", bufs=4, space="PSUM") as psp,
        ):
            gts = {}
            wtt = {}

            def issue_loads(t):
                cols = ccol0[t + 1] - ccol0[t]
                it = idxp.tile([128, cols * 8], mybir.dt.int16, tag="idx")
                nc.sync.dma_start(out=it[:], in_=gidx[:, ccol0[t] * 8: ccol0[t + 1] * 8])
                gt = gp.tile([128, cols, C], bf16, tag="gt")
                # firmware caps a single gather at 1024 indices (8 columns)
                for j0 in range(0, cols, 8):
                    gcols = min(8, cols - j0)
                    nidx = gcols * 128
                    nc.gpsimd.dma_gather(
                        out_ap=gt[:, j0:j0 + gcols], in_ap=bottom[:],
                        idxs_ap=it[:, j0 * 8: (j0 + gcols) * 8],
                        num_idxs=nidx, num_idxs_reg=nidx,
                        elem_size=C,
                    )
                wt = wp.tile([128, cols, 2, 2, 49], bf16, tag="wt")
                nc.sync.dma_start(out=wt[:], in_=wts[:, ccol0[t]: ccol0[t + 1]])
                gts[t] = gt
                wtt[t] = wt

            issue_loads(0)
            if _DBG_NCHUNK > 1:
                issue_loads(1)
            pair = None
            for t in range(_DBG_NCHUNK):
                if t + 2 < _DBG_NCHUNK:
                    issue_loads(t + 2)
                gt = gts.pop(t)
                wt = wtt.pop(t)
                for kk in range(SLOTS_PER_CHUNK):
                    ei = t * SLOTS_PER_CHUNK + kk   # emission index
                    k = plan["chunk_slots"][t][kk]
                    rl = runs[k]
                    nrun = len(rl)
                    psE = psp.tile([128, 4, 2, 49], f32, tag="psE")
                    psO = psp.tile([128, 4, 2, 49], f32, tag="psO")
                    for cc in range(4):
                        for ri, (g, p0, plen) in enumerate(rl):
                            lhs = gt[p0:p0 + plen, g, cc * 128:(cc + 1) * 128]
                            st = ri == 0
                            sp = ri == nrun - 1
                            nc.tensor.matmul(out=psE[:, cc], lhsT=lhs,
                                             rhs=wt[p0:p0 + plen, g, 0],
                                             start=st, stop=sp)
                            nc.tensor.matmul(out=psO[:, cc], lhsT=lhs,
                                             rhs=wt[p0:p0 + plen, g, 1],
                                             start=st, stop=sp)
                    sbE = ep.tile([128, 4, 2, 49], f32, tag="sbE")
                    nc.scalar.copy(out=sbE[:], in_=psE[:])
                    v = vp.tile([128, 4, 2, 49], bf16, tag="v")
                    nc.vector.tensor_tensor(out=v[:], in0=psO[:], in1=sbE[:], op=mmax)
                    if ei % 2 == 0:
                        pair = pp.tile([128, 2, 4, 49], bf16, tag="pair")
                    nc.vector.tensor_tensor(out=pair[:, ei % 2], in0=v[:, :, 0],
                                            in1=v[:, :, 1], op=mmax)
                    if ei % 2 == 1:
                        nc.scalar.dma_start(out=out[ei // 2], in_=pair[:])
    nc.compile()
    _NC_CACHE[key] = nc
    return nc


def _build_core_inputs(g, roi_ids, plan, flat_bf16):
    totq = plan["totq"]
    totcols = plan["chunk_col0"][NCHUNK]
    gq = np.zeros(totq, np.int16)
    wt = np.zeros((128, totcols, 196), np.float32)
    for k, n in enumerate(roi_ids):
        bid, rmin, cmin, nrows, ncols, Wfull = _roi_weights(g, n)
        pts = nrows * ncols
        rr, xx = np.divmod(np.arange(pts), ncols)
        flat_idx = (bid * (H * W) + (rmin + rr) * W + (cmin + xx)).astype(np.int16)
        qs = plan["slot_q0"][k] + np.arange(pts)
        gq[qs] = flat_idx
        Wsel = Wfull[_PERM_FLAT]                     # [196 (e,dj,pp), pts]
        wt[qs % 128, qs // 128, :] = Wsel.T
    it = np.empty((16, totq // 16), np.int16)
    qpos = np.arange(totq)
    it[qpos % 16, qpos // 16] = gq
    return {
        "bottom": flat_bf16,
        "gidx": np.tile(it, (8, 1)),
        "w": wt.reshape(128, totcols, 2, 2, 49).astype(BF16),
    }


def _prepare(bottom, rois, im_info):
    g = _grid_geometry(rois, im_info)
    pts = _roi_pts(g)
    cores = _assign_rois_to_cores(pts)
    plan = _make_plan(pts, cores)
    flat = np.ascontiguousarray(
        np.asarray(bottom, np.float32).reshape(B * H * W, C)).astype(BF16)
    in_maps = [_build_core_inputs(g, cores[c], plan, flat) for c in range(NCORES)]
    return cores, plan, in_maps


def kernel(bottom, rois, im_info):
    from concourse.bass_utils import run_bass_kernel_spmd

    cores, plan, in_maps = _prepare(bottom, rois, im_info)
    nc = _build_bass(plan)
    res = run_bass_kernel_spmd(nc, in_maps, core_ids=list(range(NCORES)))
    out = np.empty((NROI, POOL, POOL, C), np.float32)
    emit = np.array(plan["emit"])
    for c in range(NCORES):
        arr = np.asarray(res.results[c]["out"], dtype=np.float32)
        # [pair, p, r2, cc, pp] -> [pair, r2, pp, cc, p] -> [NR, 49, 512]
        vals = arr.transpose(0, 2, 4, 3, 1).reshape(NR, POOL, POOL, C)
        out[np.array(cores[c])[emit]] = vals
    return out


# revision 13
# speedup vs baseline: 3.1646x; 1.2559x over previous
"""Trainium2 Bass kernel for CropPoolLayer (TF crop_and_resize bilinear + 2x2 maxpool).

Decomposition (host precomputes indices + dense per-ROI bilinear weights):
  per ROI: crops[(i,j), c] = sum_q W[q, (i,j)] * bottom_flat[pt[q], c]
  pooled  = 2x2 max over (i,j)

Device (per core, SPMD over 8 cores, 64 ROIs each, all wire data bf16):
  - per 8-ROI chunk: one batched dma_gather of all chunk points (rows of
    512 ch, 1KB bf16 each) into [128, cols, 512]; ROIs packed at
    64-partition granularity (column = 128 points).
  - transposed matmuls: lhsT = gathered data [pts, 128 c-chunk], rhs =
    weights [pts, 98] -> psum [128(c), 98] so channel is the psum
    partition dim and the (i,j) samples live on the FREE axis.
    Two psum tiles per ROI: psE (even i), psO (odd i), each [128, 4(cc),
    2(dj), 49(pi*7+pj)].
  - pool: ACT evacuates psE -> sbE; DVE max(psO, sbE) folds di; Pool
    engine folds dj on the free axis -> pooled [128, 4, 49] bf16.
  - output written as [pair, 128(c%128), 2(roi), 4(c//128), 49] bf16,
    un-transposed on host.
"""
import sys

sys.path.insert(0, "/opt/trn_rl_repo")

import numpy as np
import ml_dtypes

POOL = 7
CROP = 14
B, H, W, C = 2, 64, 64, 512
NROI = 512
NCORES = 8
NR = NROI // NCORES            # 64 ROIs per core
SLOTS_PER_CHUNK = 8
NCHUNK = NR // SLOTS_PER_CHUNK  # 8 gather chunks per core

BF16 = ml_dtypes.bfloat16

# m-order permutation: (e, dj, pi*7+pj) -> row (2*pi+e)*14 + (2*pj+dj) of Wfull
_PERM = np.empty((2, 2, 49), np.int64)
for _e in range(2):
    for _dj in range(2):
        for _pi in range(POOL):
            for _pj in range(POOL):
                _PERM[_e, _dj, _pi * 7 + _pj] = (2 * _pi + _e) * 14 + (2 * _pj + _dj)
_PERM_FLAT = _PERM.reshape(-1)


def _grid_geometry(rois, im_info):
    rois = np.asarray(rois, dtype=np.float32)
    im_h = np.float32(im_info[0])
    im_w = np.float32(im_info[1])
    bids = rois[:, 0].astype(np.int32)
    x1 = rois[:, 1] / im_w
    y1 = rois[:, 2] / im_h
    x2 = rois[:, 3] / im_w
    y2 = rois[:, 4] / im_h
    grid = np.arange(CROP, dtype=np.float32)
    ys = y1[:, None] * np.float32(H - 1) + grid[None, :] * ((y2 - y1) * np.float32(H - 1) / np.float32(CROP - 1))[:, None]
    xs = x1[:, None] * np.float32(W - 1) + grid[None, :] * ((x2 - x1) * np.float32(W - 1) / np.float32(CROP - 1))[:, None]
    vy = (ys >= 0) & (ys <= H - 1)
    vx = (xs >= 0) & (xs <= W - 1)
    y0f = np.floor(ys)
    x0f = np.floor(xs)
    ly = (ys - y0f).astype(np.float32)
    lx = (xs - x0f).astype(np.float32)
    y0 = np.clip(y0f, 0, H - 1).astype(np.int32)
    y1i = np.clip(np.ceil(ys), 0, H - 1).astype(np.int32)
    x0 = np.clip(x0f, 0, W - 1).astype(np.int32)
    x1i = np.clip(np.ceil(xs), 0, W - 1).astype(np.int32)
    return dict(bids=bids, ly=ly, lx=lx, y0=y0, y1i=y1i, x0=x0, x1i=x1i, vy=vy, vx=vx)


def _roi_weights(g, n):
    y0 = g["y0"][n]; y1i = g["y1i"][n]; x0 = g["x0"][n]; x1i = g["x1i"][n]
    ly = g["ly"][n]; lx = g["lx"][n]; vy = g["vy"][n]; vx = g["vx"][n]
    rmin = int(min(y0.min(), y1i.min())); rmax = int(max(y0.max(), y1i.max()))
    cmin = int(min(x0.min(), x1i.min())); cmax = int(max(x0.max(), x1i.max()))
    nrows = rmax - rmin + 1; ncols = cmax - cmin + 1
    Wy = np.zeros((CROP, nrows), np.float32)
    Wx = np.zeros((CROP, ncols), np.float32)
    ii = np.arange(CROP)
    np.add.at(Wy, (ii, y0 - rmin), ((1.0 - ly) * vy).astype(np.float32))
    np.add.at(Wy, (ii, y1i - rmin), (ly * vy).astype(np.float32))
    np.add.at(Wx, (ii, x0 - cmin), ((1.0 - lx) * vx).astype(np.float32))
    np.add.at(Wx, (ii, x1i - cmin), (lx * vx).astype(np.float32))
    Wfull = np.einsum("ir,jx->ijrx", Wy, Wx).reshape(CROP * CROP, nrows * ncols)
    return int(g["bids"][n]), rmin, cmin, nrows, ncols, Wfull


def _roi_pts(g):
    pts = np.empty(NROI, np.int64)
    for n in range(NROI):
        y0 = g["y0"][n]; y1i = g["y1i"][n]; x0 = g["x0"][n]; x1i = g["x1i"][n]
        nrows = int(max(y0.max(), y1i.max())) - int(min(y0.min(), y1i.min())) + 1
        ncols = int(max(x0.max(), x1i.max())) - int(min(x0.min(), x1i.min())) + 1
        pts[n] = nrows * ncols
    return pts


def _assign_rois_to_cores(pts):
    """Sort ascending by pts, snake-distribute; per-core slot lists end up
    ascending so cross-core slot maxima stay tight."""
    order = np.argsort(pts, kind="stable")
    cores = [[] for _ in range(NCORES)]
    for rank, n in enumerate(order):
        rnd, pos = divmod(rank, NCORES)
        c = pos if rnd % 2 == 0 else NCORES - 1 - pos
        cores[c].append(int(n))
    return cores


def _make_plan(pts, cores):
    """Shared (cross-core) gather/compute layout.

    Every slot k gets span64[k] = roundup64(max_core pts) point positions,
    packed sequentially; each 8-slot chunk starts on a 128 (column)
    boundary. Returns per-slot q-offsets, per-chunk column ranges and
    per-slot matmul runs (column, partition base, length)."""
    span = [0] * NR
    for k in range(NR):
        m = max(pts[cores[c][k]] for c in range(NCORES))
        span[k] = ((int(m) + 63) // 64) * 64
    # stride-interleave sorted slots into chunks so each chunk mixes small
    # and large ROIs (balances per-chunk gather DMA against compute)
    chunk_slots = [[t + NCHUNK * j for j in range(SLOTS_PER_CHUNK)]
                   for t in range(NCHUNK)]
    emit = [k for cs in chunk_slots for k in cs]
    chunk_of = {}
    for t, cs in enumerate(chunk_slots):
        for k in cs:
            chunk_of[k] = t
    slot_q0 = [0] * NR
    chunk_col0 = [0] * (NCHUNK + 1)
    q = 0
    for t in range(NCHUNK):
        assert q % 128 == 0
        chunk_col0[t] = q // 128
        for k in chunk_slots[t]:
            # HW: psum accumulation groups require base-0 matmul operands, so
            # multi-run (span>64) slots must start on a column boundary; only
            # span-64 slots may sit at offset 64 (their single matmul at base
            # 64 is fine).
            if span[k] > 64 and q % 128 != 0:
                q += 64
            slot_q0[k] = q
            q += span[k]
        q = ((q + 127) // 128) * 128
    chunk_col0[NCHUNK] = q // 128
    totq = q
    runs = []
    for k in range(NR):
        rl = []
        s = slot_q0[k] - chunk_col0[chunk_of[k]] * 128
        rem = span[k]
        while rem > 0:
            p0 = s % 128
            g = s // 128
            plen = min(rem, 128 - p0)
            rl.append((g, p0, plen))
            s += plen
            rem -= plen
        runs.append(rl)
    return dict(span=span, slot_q0=slot_q0, chunk_col0=chunk_col0, totq=totq,
                runs=tuple(tuple(r) for r in runs), emit=tuple(emit),
                chunk_slots=tuple(tuple(cs) for cs in chunk_slots))


_NC_CACHE = {}
_DBG_NCHUNK = NCHUNK  # debug knob: build only the first N chunks


def _build_bass(plan):
    key = (plan["totq"], tuple(plan["chunk_col0"]), plan["runs"], _DBG_NCHUNK)
    if key in _NC_CACHE:
        return _NC_CACHE[key]
    import concourse.bacc as bacc
    import concourse.mybir as mybir
    from concourse.tile import TileContext

    f32 = mybir.dt.float32
    bf16 = mybir.dt.bfloat16
    mmax = mybir.AluOpType.max

    totcols = plan["chunk_col0"][NCHUNK]
    ccol0 = plan["chunk_col0"]
    runs = plan["runs"]

    nc = bacc.Bacc("TRN2", target_bir_lowering=False, debug=False, num_devices=NCORES)
    bottom = nc.dram_tensor("bottom", [B * H * W, C], bf16, kind="ExternalInput")
    gidx = nc.dram_tensor("gidx", [128, plan["totq"] // 16], mybir.dt.int16, kind="ExternalInput")
    wts = nc.dram_tensor("w", [128, totcols, 2, 2, 49], bf16, kind="ExternalInput")
    out = nc.dram_tensor("out", [NR // 2, 128, 2, 4, 49], bf16, kind="ExternalOutput")

    with TileContext(nc) as tc:
        with (
            tc.tile_pool(name="idxp", bufs=3) as idxp,
            tc.tile_pool(name="gp", bufs=3) as gp,
            tc.tile_pool(name="wp", bufs=3) as wp,
            tc.tile_pool(name="ep", bufs=6) as ep,
            tc.tile_pool(name="vp", bufs=6) as vp,
            tc.tile_pool(name="pp", bufs=4) as pp,
            tc.tile_pool(name="psp", bufs=4, space="PSUM") as psp,
        ):
            gts = {}
            wtt = {}

            def issue_loads(t):
                cols = ccol0[t + 1] - ccol0[t]
                it = idxp.tile([128, cols * 8], mybir.dt.int16, tag="idx")
                nc.sync.dma_start(out=it[:], in_=gidx[:, ccol0[t] * 8: ccol0[t + 1] * 8])
                gt = gp.tile([128, cols, C], bf16, tag="gt")
                # firmware caps a single gather at 1024 indices; use 4-col
                # pieces so short DMAs can interleave on the DMA engines
                for j0 in range(0, cols, 4):
                    gcols = min(4, cols - j0)
                    nidx = gcols * 128
                    nc.gpsimd.dma_gather(
                        out_ap=gt[:, j0:j0 + gcols], in_ap=bottom[:],
                        idxs_ap=it[:, j0 * 8: (j0 + gcols) * 8],
                        num_idxs=nidx, num_idxs_reg=nidx,
                        elem_size=C,
                    )
                wt = wp.tile([128, cols, 2, 2, 49], bf16, tag="wt")
                nc.sync.dma_start(out=wt[:], in_=wts[:, ccol0[t]: ccol0[t + 1]])
                gts[t] = gt
                wtt[t] = wt

            issue_loads(0)
            if _DBG_NCHUNK > 1:
                issue_loads(1)
            pair = None
            for t in range(_DBG_NCHUNK):
                if t + 2 < _DBG_NCHUNK:
                    issue_loads(t + 2)
                gt = gts.pop(t)
                wt = wtt.pop(t)
                for kk in range(SLOTS_PER_CHUNK):
                    ei = t * SLOTS_PER_CHUNK + kk   # emission index
                    k = plan["chunk_slots"][t][kk]
                    rl = runs[k]
                    nrun = len(rl)
                    psE = psp.tile([128, 4, 2, 49], f32, tag="psE")
                    psO = psp.tile([128, 4, 2, 49], f32, tag="psO")
                    for cc in range(4):
                        for ri, (g, p0, plen) in enumerate(rl):
                            lhs = gt[p0:p0 + plen, g, cc * 128:(cc + 1) * 128]
                            st = ri == 0
                            sp = ri == nrun - 1
                            nc.tensor.matmul(out=psE[:, cc], lhsT=lhs,
                                             rhs=wt[p0:p0 + plen, g, 0],
                                             start=st, stop=sp)
                            nc.tensor.matmul(out=psO[:, cc], lhsT=lhs,
                                             rhs=wt[p0:p0 + plen, g, 1],
                                             start=st, stop=sp)
                    sbE = ep.tile([128, 4, 2, 49], f32, tag="sbE")
                    nc.scalar.copy(out=sbE[:], in_=psE[:])
                    v = vp.tile([128, 4, 2, 49], bf16, tag="v")
                    nc.vector.tensor_tensor(out=v[:], in0=psO[:], in1=sbE[:], op=mmax)
                    if ei % 2 == 0:
                        pair = pp.tile([128, 2, 4, 49], bf16, tag="pair")
                    nc.vector.tensor_tensor(out=pair[:, ei % 2], in0=v[:, :, 0],
                                            in1=v[:, :, 1], op=mmax)
                    if ei % 2 == 1:
                        nc.scalar.dma_start(out=out[ei // 2], in_=pair[:])
    nc.compile()
    _NC_CACHE[key] = nc
    return nc


def _build_core_inputs(g, roi_ids, plan, flat_bf16):
    totq = plan["totq"]
    totcols = plan["chunk_col0"][NCHUNK]
    gq = np.zeros(totq, np.int16)
    wt = np.zeros((128, totcols, 196), np.float32)
    for k, n in enumerate(roi_ids):
        bid, rmin, cmin, nrows, ncols, Wfull = _roi_weights(g, n)
        pts = nrows * ncols
        rr, xx = np.divmod(np.arange(pts), ncols)
        flat_idx = (bid * (H * W) + (rmin + rr) * W + (cmin + xx)).astype(np.int16)
        qs = plan["slot_q0"][k] + np.arange(pts)
        gq[qs] = flat_idx
        Wsel = Wfull[_PERM_FLAT]                     # [196 (e,dj,pp), pts]
        wt[qs % 128, qs // 128, :] = Wsel.T
    it = np.empty((16, totq // 16), np.int16)
    qpos = np.arange(totq)
    it[qpos % 16, qpos // 16] = gq
    return {
        "bottom": flat_bf16,
        "gidx": np.tile(it, (8, 1)),
        "w": wt.reshape(128, totcols, 2, 2, 49).astype(BF16),
    }


def _prepare(bottom, rois, im_info):
    g = _grid_geometry(rois, im_info)
    pts = _roi_pts(g)
    cores = _assign_rois_to_cores(pts)
    plan = _make_plan(pts, cores)
    flat = np.ascontiguousarray(
        np.asarray(bottom, np.float32).reshape(B * H * W, C)).astype(BF16)
    in_maps = [_build_core_inputs(g, cores[c], plan, flat) for c in range(NCORES)]
    return cores, plan, in_maps


def kernel(bottom, rois, im_info):
    from concourse.bass_utils import run_bass_kernel_spmd

    cores, plan, in_maps = _prepare(bottom, rois, im_info)
    nc = _build_bass(plan)
    res = run_bass_kernel_spmd(nc, in_maps, core_ids=list(range(NCORES)))
    out = np.empty((NROI, POOL, POOL, C), np.float32)
    emit = np.array(plan["emit"])
    for c in range(NCORES):
        arr = np.asarray(res.results[c]["out"], dtype=np.float32)
        # [pair, p, r2, cc, pp] -> [pair, r2, pp, cc, p] -> [NR, 49, 512]
        vals = arr.transpose(0, 2, 4, 3, 1).reshape(NR, POOL, POOL, C)
        out[np.array(cores[c])[emit]] = vals
    return out


# revision 16
# speedup vs baseline: 3.1717x; 1.0023x over previous
"""Trainium2 Bass kernel for CropPoolLayer (TF crop_and_resize bilinear + 2x2 maxpool).

Decomposition (host precomputes indices + dense per-ROI bilinear weights):
  per ROI: crops[(i,j), c] = sum_q W[q, (i,j)] * bottom_flat[pt[q], c]
  pooled  = 2x2 max over (i,j)

Device (per core, SPMD over 8 cores, 64 ROIs each, all wire data bf16):
  - per 8-ROI chunk: one batched dma_gather of all chunk points (rows of
    512 ch, 1KB bf16 each) into [128, cols, 512]; ROIs packed at
    64-partition granularity (column = 128 points).
  - transposed matmuls: lhsT = gathered data [pts, 128 c-chunk], rhs =
    weights [pts, 98] -> psum [128(c), 98] so channel is the psum
    partition dim and the (i,j) samples live on the FREE axis.
    Two psum tiles per ROI: psE (even i), psO (odd i), each [128, 4(cc),
    2(dj), 49(pi*7+pj)].
  - pool: ACT evacuates psE -> sbE; DVE max(psO, sbE) folds di; Pool
    engine folds dj on the free axis -> pooled [128, 4, 49] bf16.
  - output written as [pair, 128(c%128), 2(roi), 4(c//128), 49] bf16,
    un-transposed on host.
"""
import sys

sys.path.insert(0, "/opt/trn_rl_repo")

import numpy as np
import ml_dtypes

POOL = 7
CROP = 14
B, H, W, C = 2, 64, 64, 512
NROI = 512
NCORES = 8
NR = NROI // NCORES            # 64 ROIs per core
SLOTS_PER_CHUNK = 8
NCHUNK = NR // SLOTS_PER_CHUNK  # 8 gather chunks per core

BF16 = ml_dtypes.bfloat16

# m-order permutation: (e, dj, pi*7+pj) -> row (2*pi+e)*14 + (2*pj+dj) of Wfull
_PERM = np.empty((2, 2, 49), np.int64)
for _e in range(2):
    for _dj in range(2):
        for _pi in range(POOL):
            for _pj in range(POOL):
                _PERM[_e, _dj, _pi * 7 + _pj] = (2 * _pi + _e) * 14 + (2 * _pj + _dj)
_PERM_FLAT = _PERM.reshape(-1)


def _grid_geometry(rois, im_info):
    rois = np.asarray(rois, dtype=np.float32)
    im_h = np.float32(im_info[0])
    im_w = np.float32(im_info[1])
    bids = rois[:, 0].astype(np.int32)
    x1 = rois[:, 1] / im_w
    y1 = rois[:, 2] / im_h
    x2 = rois[:, 3] / im_w
    y2 = rois[:, 4] / im_h
    grid = np.arange(CROP, dtype=np.float32)
    ys = y1[:, None] * np.float32(H - 1) + grid[None, :] * ((y2 - y1) * np.float32(H - 1) / np.float32(CROP - 1))[:, None]
    xs = x1[:, None] * np.float32(W - 1) + grid[None, :] * ((x2 - x1) * np.float32(W - 1) / np.float32(CROP - 1))[:, None]
    vy = (ys >= 0) & (ys <= H - 1)
    vx = (xs >= 0) & (xs <= W - 1)
    y0f = np.floor(ys)
    x0f = np.floor(xs)
    ly = (ys - y0f).astype(np.float32)
    lx = (xs - x0f).astype(np.float32)
    y0 = np.clip(y0f, 0, H - 1).astype(np.int32)
    y1i = np.clip(np.ceil(ys), 0, H - 1).astype(np.int32)
    x0 = np.clip(x0f, 0, W - 1).astype(np.int32)
    x1i = np.clip(np.ceil(xs), 0, W - 1).astype(np.int32)
    return dict(bids=bids, ly=ly, lx=lx, y0=y0, y1i=y1i, x0=x0, x1i=x1i, vy=vy, vx=vx)


def _roi_weights(g, n):
    y0 = g["y0"][n]; y1i = g["y1i"][n]; x0 = g["x0"][n]; x1i = g["x1i"][n]
    ly = g["ly"][n]; lx = g["lx"][n]; vy = g["vy"][n]; vx = g["vx"][n]
    rmin = int(min(y0.min(), y1i.min())); rmax = int(max(y0.max(), y1i.max()))
    cmin = int(min(x0.min(), x1i.min())); cmax = int(max(x0.max(), x1i.max()))
    nrows = rmax - rmin + 1; ncols = cmax - cmin + 1
    Wy = np.zeros((CROP, nrows), np.float32)
    Wx = np.zeros((CROP, ncols), np.float32)
    ii = np.arange(CROP)
    np.add.at(Wy, (ii, y0 - rmin), ((1.0 - ly) * vy).astype(np.float32))
    np.add.at(Wy, (ii, y1i - rmin), (ly * vy).astype(np.float32))
    np.add.at(Wx, (ii, x0 - cmin), ((1.0 - lx) * vx).astype(np.float32))
    np.add.at(Wx, (ii, x1i - cmin), (lx * vx).astype(np.float32))
    Wfull = np.einsum("ir,jx->ijrx", Wy, Wx).reshape(CROP * CROP, nrows * ncols)
    return int(g["bids"][n]), rmin, cmin, nrows, ncols, Wfull


def _roi_pts(g):
    pts = np.empty(NROI, np.int64)
    for n in range(NROI):
        y0 = g["y0"][n]; y1i = g["y1i"][n]; x0 = g["x0"][n]; x1i = g["x1i"][n]
        nrows = int(max(y0.max(), y1i.max())) - int(min(y0.min(), y1i.min())) + 1
        ncols = int(max(x0.max(), x1i.max())) - int(min(x0.min(), x1i.min())) + 1
        pts[n] = nrows * ncols
    return pts


def _assign_rois_to_cores(pts):
    """Sort ascending by pts, snake-distribute; per-core slot lists end up
    ascending so cross-core slot maxima stay tight."""
    order = np.argsort(pts, kind="stable")
    cores = [[] for _ in range(NCORES)]
    for rank, n in enumerate(order):
        rnd, pos = divmod(rank, NCORES)
        c = pos if rnd % 2 == 0 else NCORES - 1 - pos
        cores[c].append(int(n))
    return cores


def _make_plan(pts, cores):
    """Shared (cross-core) gather/compute layout.

    Every slot k gets span64[k] = roundup64(max_core pts) point positions,
    packed sequentially; each 8-slot chunk starts on a 128 (column)
    boundary. Returns per-slot q-offsets, per-chunk column ranges and
    per-slot matmul runs (column, partition base, length)."""
    span = [0] * NR
    for k in range(NR):
        m = max(pts[cores[c][k]] for c in range(NCORES))
        span[k] = ((int(m) + 31) // 32) * 32
    # stride-interleave sorted slots into chunks so each chunk mixes small
    # and large ROIs (balances per-chunk gather DMA against compute)
    chunk_slots = [[t + NCHUNK * j for j in range(SLOTS_PER_CHUNK)]
                   for t in range(NCHUNK)]
    emit = [k for cs in chunk_slots for k in cs]
    chunk_of = {}
    for t, cs in enumerate(chunk_slots):
        for k in cs:
            chunk_of[k] = t
    slot_q0 = [0] * NR
    chunk_col0 = [0] * (NCHUNK + 1)
    q = 0
    for t in range(NCHUNK):
        assert q % 128 == 0
        chunk_col0[t] = q // 128
        for k in chunk_slots[t]:
            # HW rules: (a) psum accumulation groups (multi-run slots) need
            # base-0 matmul operands -> column-aligned; (b) single-run APs
            # >32 partitions must start at base 0/32/64 -> offsets restricted
            # to {0, 32, 64} within a column.
            o = q % 128
            if span[k] + o > 128:          # would need multiple runs
                if o != 0:
                    q += 128 - o           # column-align
            elif o not in (0, 32, 64):     # offset 96: advance to next column
                q += 128 - o
            slot_q0[k] = q
            q += span[k]
        q = ((q + 127) // 128) * 128
    chunk_col0[NCHUNK] = q // 128
    totq = q
    runs = []
    for k in range(NR):
        rl = []
        s = slot_q0[k] - chunk_col0[chunk_of[k]] * 128
        rem = span[k]
        while rem > 0:
            p0 = s % 128
            g = s // 128
            plen = min(rem, 128 - p0)
            rl.append((g, p0, plen))
            s += plen
            rem -= plen
        runs.append(rl)
    return dict(span=span, slot_q0=slot_q0, chunk_col0=chunk_col0, totq=totq,
                runs=tuple(tuple(r) for r in runs), emit=tuple(emit),
                chunk_slots=tuple(tuple(cs) for cs in chunk_slots))


_NC_CACHE = {}
_DBG_NCHUNK = NCHUNK  # debug knob: build only the first N chunks


def _build_bass(plan):
    key = (plan["totq"], tuple(plan["chunk_col0"]), plan["runs"], _DBG_NCHUNK)
    if key in _NC_CACHE:
        return _NC_CACHE[key]
    import concourse.bacc as bacc
    import concourse.mybir as mybir
    from concourse.tile import TileContext

    f32 = mybir.dt.float32
    bf16 = mybir.dt.bfloat16
    mmax = mybir.AluOpType.max

    totcols = plan["chunk_col0"][NCHUNK]
    ccol0 = plan["chunk_col0"]
    runs = plan["runs"]

    nc = bacc.Bacc("TRN2", target_bir_lowering=False, debug=False, num_devices=NCORES)
    bottom = nc.dram_tensor("bottom", [B * H * W, C], bf16, kind="ExternalInput")
    gidx = nc.dram_tensor("gidx", [128, plan["totq"] // 16], mybir.dt.int16, kind="ExternalInput")
    wts = nc.dram_tensor("w", [128, totcols, 2, 2, 49], bf16, kind="ExternalInput")
    out = nc.dram_tensor("out", [NR // 2, 128, 2, 4, 49], bf16, kind="ExternalOutput")

    with TileContext(nc) as tc:
        with (
            tc.tile_pool(name="idxp", bufs=3) as idxp,
            tc.tile_pool(name="gp", bufs=3) as gp,
            tc.tile_pool(name="wp", bufs=3) as wp,
            tc.tile_pool(name="ep", bufs=6) as ep,
            tc.tile_pool(name="vp", bufs=6) as vp,
            tc.tile_pool(name="pp", bufs=4) as pp,
            tc.tile_pool(name="psp", bufs=4, space="PSUM") as psp,
        ):
            gts = {}
            wtt = {}

            def issue_loads(t):
                cols = ccol0[t + 1] - ccol0[t]
                it = idxp.tile([128, cols * 8], mybir.dt.int16, tag="idx")
                nc.sync.dma_start(out=it[:], in_=gidx[:, ccol0[t] * 8: ccol0[t + 1] * 8])
                gt = gp.tile([128, cols, C], bf16, tag="gt")
                # firmware caps a single gather at 1024 indices; use 4-col
                # pieces so short DMAs can interleave on the DMA engines.
                # chunk 0 leads with a 1-col piece to cut the startup latency
                # to the first matmul.
                j0 = 0
                while j0 < cols:
                    gcols = 1 if (t == 0 and j0 == 0) else min(4, cols - j0)
                    nidx = gcols * 128
                    nc.gpsimd.dma_gather(
                        out_ap=gt[:, j0:j0 + gcols], in_ap=bottom[:],
                        idxs_ap=it[:, j0 * 8: (j0 + gcols) * 8],
                        num_idxs=nidx, num_idxs_reg=nidx,
                        elem_size=C,
                    )
                    j0 += gcols
                wt = wp.tile([128, cols, 2, 2, 49], bf16, tag="wt")
                nc.sync.dma_start(out=wt[:], in_=wts[:, ccol0[t]: ccol0[t + 1]])
                gts[t] = gt
                wtt[t] = wt

            issue_loads(0)
            if _DBG_NCHUNK > 1:
                issue_loads(1)
            pair = None
            for t in range(_DBG_NCHUNK):
                if t + 2 < _DBG_NCHUNK:
                    issue_loads(t + 2)
                gt = gts.pop(t)
                wt = wtt.pop(t)
                for kk in range(SLOTS_PER_CHUNK):
                    ei = t * SLOTS_PER_CHUNK + kk   # emission index
                    k = plan["chunk_slots"][t][kk]
                    rl = runs[k]
                    nrun = len(rl)
                    psE = psp.tile([128, 4, 2, 49], f32, tag="psE")
                    psO = psp.tile([128, 4, 2, 49], f32, tag="psO")
                    for cc in range(4):
                        for ri, (g, p0, plen) in enumerate(rl):
                            lhs = gt[p0:p0 + plen, g, cc * 128:(cc + 1) * 128]
                            st = ri == 0
                            sp = ri == nrun - 1
                            nc.tensor.matmul(out=psE[:, cc], lhsT=lhs,
                                             rhs=wt[p0:p0 + plen, g, 0],
                                             start=st, stop=sp)
                            nc.tensor.matmul(out=psO[:, cc], lhsT=lhs,
                                             rhs=wt[p0:p0 + plen, g, 1],
                                             start=st, stop=sp)
                    sbE = ep.tile([128, 4, 2, 49], f32, tag="sbE")
                    nc.scalar.copy(out=sbE[:], in_=psE[:])
                    v = vp.tile([128, 4, 2, 49], bf16, tag="v")
                    nc.vector.tensor_tensor(out=v[:], in0=psO[:], in1=sbE[:], op=mmax)
                    if ei % 2 == 0:
                        pair = pp.tile([128, 2, 4, 49], bf16, tag="pair")
                    nc.vector.tensor_tensor(out=pair[:, ei % 2], in0=v[:, :, 0],
                                            in1=v[:, :, 1], op=mmax)
                    if ei % 2 == 1:
                        nc.scalar.dma_start(out=out[ei // 2], in_=pair[:])
    nc.compile()
    _NC_CACHE[key] = nc
    return nc


def _build_core_inputs(g, roi_ids, plan, flat_bf16):
    totq = plan["totq"]
    totcols = plan["chunk_col0"][NCHUNK]
    gq = np.zeros(totq, np.int16)
    wt = np.zeros((128, totcols, 196), np.float32)
    for k, n in enumerate(roi_ids):
        bid, rmin, cmin, nrows, ncols, Wfull = _roi_weights(g, n)
        pts = nrows * ncols
        rr, xx = np.divmod(np.arange(pts), ncols)
        flat_idx = (bid * (H * W) + (rmin + rr) * W + (cmin + xx)).astype(np.int16)
        qs = plan["slot_q0"][k] + np.arange(pts)
        gq[qs] = flat_idx
        Wsel = Wfull[_PERM_FLAT]                     # [196 (e,dj,pp), pts]
        wt[qs % 128, qs // 128, :] = Wsel.T
    it = np.empty((16, totq // 16), np.int16)
    qpos = np.arange(totq)
    it[qpos % 16, qpos // 16] = gq
    return {
        "bottom": flat_bf16,
        "gidx": np.tile(it, (8, 1)),
        "w": wt.reshape(128, totcols, 2, 2, 49).astype(BF16),
    }


def _prepare(bottom, rois, im_info):
    g = _grid_geometry(rois, im_info)
    pts = _roi_pts(g)
    cores = _assign_rois_to_cores(pts)
    plan = _make_plan(pts, cores)
    flat = np.ascontiguousarray(
        np.asarray(bottom, np.float32).reshape(B * H * W, C)).astype(BF16)
    in_maps = [_build_core_inputs(g, cores[c], plan, flat) for c in range(NCORES)]
    return cores, plan, in_maps


def kernel(bottom, rois, im_info):
    from concourse.bass_utils import run_bass_kernel_spmd

    cores, plan, in_maps = _prepare(bottom, rois, im_info)
    nc = _build_bass(plan)
    res = run_bass_kernel_spmd(nc, in_maps, core_ids=list(range(NCORES)))
    out = np.empty((NROI, POOL, POOL, C), np.float32)
    emit = np.array(plan["emit"])
    for c in range(NCORES):
        arr = np.asarray(res.results[c]["out"], dtype=np.float32)
        # [pair, p, r2, cc, pp] -> [pair, r2, pp, cc, p] -> [NR, 49, 512]
        vals = arr.transpose(0, 2, 4, 3, 1).reshape(NR, POOL, POOL, C)
        out[np.array(cores[c])[emit]] = vals
    return out


# revision 19
# speedup vs baseline: 3.3842x; 1.0670x over previous
"""Trainium2 Bass kernel for CropPoolLayer (TF crop_and_resize bilinear + 2x2 maxpool).

Decomposition (host precomputes indices + dense per-ROI bilinear weights):
  per ROI: crops[(i,j), c] = sum_q W[q, (i,j)] * bottom_flat[pt[q], c]
  pooled  = 2x2 max over (i,j)

Device (per core, SPMD over 8 cores, 64 ROIs each, all wire data bf16):
  - per 8-ROI chunk: one batched dma_gather of all chunk points (rows of
    512 ch, 1KB bf16 each) into [128, cols, 512]; ROIs packed at
    64-partition granularity (column = 128 points).
  - transposed matmuls: lhsT = gathered data [pts, 128 c-chunk], rhs =
    weights [pts, 98] -> psum [128(c), 98] so channel is the psum
    partition dim and the (i,j) samples live on the FREE axis.
    Two psum tiles per ROI: psE (even i), psO (odd i), each [128, 4(cc),
    2(dj), 49(pi*7+pj)].
  - pool: ACT evacuates psE -> sbE; DVE max(psO, sbE) folds di; Pool
    engine folds dj on the free axis -> pooled [128, 4, 49] bf16.
  - output written as [pair, 128(c%128), 2(roi), 4(c//128), 49] bf16,
    un-transposed on host.
"""
import sys

sys.path.insert(0, "/opt/trn_rl_repo")

import numpy as np
import ml_dtypes

POOL = 7
CROP = 14
B, H, W, C = 2, 64, 64, 512
NROI = 512
NCORES = 8
NR = NROI // NCORES            # 64 ROIs per core
SLOTS_PER_CHUNK = 8
NCHUNK = NR // SLOTS_PER_CHUNK  # 8 gather chunks per core

BF16 = ml_dtypes.bfloat16

# m-order permutation: (e, dj, pi*7+pj) -> row (2*pi+e)*14 + (2*pj+dj) of Wfull
_PERM = np.empty((2, 2, 49), np.int64)
for _e in range(2):
    for _dj in range(2):
        for _pi in range(POOL):
            for _pj in range(POOL):
                _PERM[_e, _dj, _pi * 7 + _pj] = (2 * _pi + _e) * 14 + (2 * _pj + _dj)
_PERM_FLAT = _PERM.reshape(-1)


def _grid_geometry(rois, im_info):
    rois = np.asarray(rois, dtype=np.float32)
    im_h = np.float32(im_info[0])
    im_w = np.float32(im_info[1])
    bids = rois[:, 0].astype(np.int32)
    x1 = rois[:, 1] / im_w
    y1 = rois[:, 2] / im_h
    x2 = rois[:, 3] / im_w
    y2 = rois[:, 4] / im_h
    grid = np.arange(CROP, dtype=np.float32)
    ys = y1[:, None] * np.float32(H - 1) + grid[None, :] * ((y2 - y1) * np.float32(H - 1) / np.float32(CROP - 1))[:, None]
    xs = x1[:, None] * np.float32(W - 1) + grid[None, :] * ((x2 - x1) * np.float32(W - 1) / np.float32(CROP - 1))[:, None]
    vy = (ys >= 0) & (ys <= H - 1)
    vx = (xs >= 0) & (xs <= W - 1)
    y0f = np.floor(ys)
    x0f = np.floor(xs)
    ly = (ys - y0f).astype(np.float32)
    lx = (xs - x0f).astype(np.float32)
    y0 = np.clip(y0f, 0, H - 1).astype(np.int32)
    y1i = np.clip(np.ceil(ys), 0, H - 1).astype(np.int32)
    x0 = np.clip(x0f, 0, W - 1).astype(np.int32)
    x1i = np.clip(np.ceil(xs), 0, W - 1).astype(np.int32)
    return dict(bids=bids, ly=ly, lx=lx, y0=y0, y1i=y1i, x0=x0, x1i=x1i, vy=vy, vx=vx)


def _roi_weights(g, n):
    y0 = g["y0"][n]; y1i = g["y1i"][n]; x0 = g["x0"][n]; x1i = g["x1i"][n]
    ly = g["ly"][n]; lx = g["lx"][n]; vy = g["vy"][n]; vx = g["vx"][n]
    rmin = int(min(y0.min(), y1i.min())); rmax = int(max(y0.max(), y1i.max()))
    cmin = int(min(x0.min(), x1i.min())); cmax = int(max(x0.max(), x1i.max()))
    nrows = rmax - rmin + 1; ncols = cmax - cmin + 1
    Wy = np.zeros((CROP, nrows), np.float32)
    Wx = np.zeros((CROP, ncols), np.float32)
    ii = np.arange(CROP)
    np.add.at(Wy, (ii, y0 - rmin), ((1.0 - ly) * vy).astype(np.float32))
    np.add.at(Wy, (ii, y1i - rmin), (ly * vy).astype(np.float32))
    np.add.at(Wx, (ii, x0 - cmin), ((1.0 - lx) * vx).astype(np.float32))
    np.add.at(Wx, (ii, x1i - cmin), (lx * vx).astype(np.float32))
    Wfull = np.einsum("ir,jx->ijrx", Wy, Wx).reshape(CROP * CROP, nrows * ncols)
    return int(g["bids"][n]), rmin, cmin, nrows, ncols, Wfull


def _roi_pts(g):
    pts = np.empty(NROI, np.int64)
    for n in range(NROI):
        y0 = g["y0"][n]; y1i = g["y1i"][n]; x0 = g["x0"][n]; x1i = g["x1i"][n]
        nrows = int(max(y0.max(), y1i.max())) - int(min(y0.min(), y1i.min())) + 1
        ncols = int(max(x0.max(), x1i.max())) - int(min(x0.min(), x1i.min())) + 1
        pts[n] = nrows * ncols
    return pts


def _assign_rois_to_cores(pts):
    """Sort ascending by pts, snake-distribute; per-core slot lists end up
    ascending so cross-core slot maxima stay tight."""
    order = np.argsort(pts, kind="stable")
    cores = [[] for _ in range(NCORES)]
    for rank, n in enumerate(order):
        rnd, pos = divmod(rank, NCORES)
        c = pos if rnd % 2 == 0 else NCORES - 1 - pos
        cores[c].append(int(n))
    return cores


def _make_plan(pts, cores):
    """Shared (cross-core) gather/compute layout.

    Every slot k gets span64[k] = roundup64(max_core pts) point positions,
    packed sequentially; each 8-slot chunk starts on a 128 (column)
    boundary. Returns per-slot q-offsets, per-chunk column ranges and
    per-slot matmul runs (column, partition base, length)."""
    span = [0] * NR
    for k in range(NR):
        m = max(pts[cores[c][k]] for c in range(NCORES))
        span[k] = ((int(m) + 63) // 64) * 64
    # stride-interleave sorted slots into chunks so each chunk mixes small
    # and large ROIs (balances per-chunk gather DMA against compute)
    chunk_slots = [[t + NCHUNK * j for j in range(SLOTS_PER_CHUNK)]
                   for t in range(NCHUNK)]
    # greedy reorder within each chunk to minimize column-align waste: only
    # span-64 slots can occupy a half-open column (offset 64); multi-run
    # slots (span >= 128) must start column-aligned, and those with
    # span % 128 == 64 leave a half-column open that a span-64 slot can fill
    for t in range(NCHUNK):
        rest = sorted(chunk_slots[t], key=lambda k: span[k])
        ordered = []
        o = 0
        while rest:
            pick = None
            if o == 64:
                pick = next((k for k in rest if span[k] == 64), None)
            else:
                half_enders = [k for k in rest if span[k] >= 128 and span[k] % 128 == 64]
                if half_enders and any(span[k] == 64 for k in rest):
                    pick = half_enders[0]
                else:
                    pick = next((k for k in rest if span[k] % 128 == 0), None)
            if pick is None:
                pick = rest[0]
            rest.remove(pick)
            ordered.append(pick)
            if span[pick] + o > 128 and o != 0:
                o = 0
            o = (o + span[pick]) % 128
        chunk_slots[t] = ordered
    emit = [k for cs in chunk_slots for k in cs]
    chunk_of = {}
    for t, cs in enumerate(chunk_slots):
        for k in cs:
            chunk_of[k] = t
    slot_q0 = [0] * NR
    chunk_col0 = [0] * (NCHUNK + 1)
    q = 0
    for t in range(NCHUNK):
        assert q % 128 == 0
        chunk_col0[t] = q // 128
        for k in chunk_slots[t]:
            # HW rules: (a) psum accumulation groups (multi-run slots) need
            # base-0 matmul operands -> column-aligned; (b) single-run APs
            # >32 partitions must start at base 0/32/64 -> offsets restricted
            # to {0, 32, 64} within a column.
            o = q % 128
            if span[k] + o > 128:          # would need multiple runs
                if o != 0:
                    q += 128 - o           # column-align
            slot_q0[k] = q
            q += span[k]
        q = ((q + 127) // 128) * 128
    chunk_col0[NCHUNK] = q // 128
    totq = q
    runs = []
    for k in range(NR):
        rl = []
        s = slot_q0[k] - chunk_col0[chunk_of[k]] * 128
        rem = span[k]
        while rem > 0:
            p0 = s % 128
            g = s // 128
            plen = min(rem, 128 - p0)
            rl.append((g, p0, plen))
            s += plen
            rem -= plen
        runs.append(rl)
    return dict(span=span, slot_q0=slot_q0, chunk_col0=chunk_col0, totq=totq,
                runs=tuple(tuple(r) for r in runs), emit=tuple(emit),
                chunk_slots=tuple(tuple(cs) for cs in chunk_slots))


_NC_CACHE = {}
_DBG_NCHUNK = NCHUNK  # debug knob: build only the first N chunks


def _build_bass(plan):
    key = (plan["totq"], tuple(plan["chunk_col0"]), plan["runs"], _DBG_NCHUNK)
    if key in _NC_CACHE:
        return _NC_CACHE[key]
    import concourse.bacc as bacc
    import concourse.mybir as mybir
    from concourse.tile import TileContext

    f32 = mybir.dt.float32
    bf16 = mybir.dt.bfloat16
    mmax = mybir.AluOpType.max

    totcols = plan["chunk_col0"][NCHUNK]
    ccol0 = plan["chunk_col0"]
    runs = plan["runs"]

    nc = bacc.Bacc("TRN2", target_bir_lowering=False, debug=False, num_devices=NCORES)
    bottom = nc.dram_tensor("bottom", [B * H * W, C], bf16, kind="ExternalInput")
    gidx = nc.dram_tensor("gidx", [128, plan["totq"] // 16], mybir.dt.int16, kind="ExternalInput")
    wts = nc.dram_tensor("w", [128, totcols, 2, 2, 49], bf16, kind="ExternalInput")
    out = nc.dram_tensor("out", [NR // 2, 128, 2, 4, 49], bf16, kind="ExternalOutput")

    with TileContext(nc) as tc:
        with (
            tc.tile_pool(name="idxp", bufs=3) as idxp,
            tc.tile_pool(name="gp", bufs=3) as gp,
            tc.tile_pool(name="wp", bufs=3) as wp,
            tc.tile_pool(name="ep", bufs=6) as ep,
            tc.tile_pool(name="vp", bufs=6) as vp,
            tc.tile_pool(name="pp", bufs=4) as pp,
            tc.tile_pool(name="psp", bufs=4, space="PSUM") as psp,
        ):
            gts = {}
            wtt = {}

            def issue_loads(t):
                cols = ccol0[t + 1] - ccol0[t]
                it = idxp.tile([128, cols * 8], mybir.dt.int16, tag="idx")
                nc.sync.dma_start(out=it[:], in_=gidx[:, ccol0[t] * 8: ccol0[t + 1] * 8])
                gt = gp.tile([128, cols, C], bf16, tag="gt")
                # firmware caps a single gather at 1024 indices; use 4-col
                # pieces so short DMAs can interleave on the DMA engines.
                # chunk 0 leads with a 1-col piece to cut the startup latency
                # to the first matmul.
                j0 = 0
                while j0 < cols:
                    gcols = 1 if (t == 0 and j0 == 0) else min(4, cols - j0)
                    nidx = gcols * 128
                    nc.gpsimd.dma_gather(
                        out_ap=gt[:, j0:j0 + gcols], in_ap=bottom[:],
                        idxs_ap=it[:, j0 * 8: (j0 + gcols) * 8],
                        num_idxs=nidx, num_idxs_reg=nidx,
                        elem_size=C,
                    )
                    j0 += gcols
                wt = wp.tile([128, cols, 2, 2, 49], bf16, tag="wt")
                nc.sync.dma_start(out=wt[:], in_=wts[:, ccol0[t]: ccol0[t + 1]])
                gts[t] = gt
                wtt[t] = wt

            issue_loads(0)
            if _DBG_NCHUNK > 1:
                issue_loads(1)
            pair = None
            for t in range(_DBG_NCHUNK):
                if t + 2 < _DBG_NCHUNK:
                    issue_loads(t + 2)
                gt = gts.pop(t)
                wt = wtt.pop(t)
                for kk in range(SLOTS_PER_CHUNK):
                    ei = t * SLOTS_PER_CHUNK + kk   # emission index
                    k = plan["chunk_slots"][t][kk]
                    rl = runs[k]
                    nrun = len(rl)
                    psE = psp.tile([128, 4, 2, 49], f32, tag="psE")
                    psO = psp.tile([128, 4, 2, 49], f32, tag="psO")
                    for cc in range(4):
                        for ri, (g, p0, plen) in enumerate(rl):
                            lhs = gt[p0:p0 + plen, g, cc * 128:(cc + 1) * 128]
                            st = ri == 0
                            sp = ri == nrun - 1
                            nc.tensor.matmul(out=psE[:, cc], lhsT=lhs,
                                             rhs=wt[p0:p0 + plen, g, 0],
                                             start=st, stop=sp)
                            nc.tensor.matmul(out=psO[:, cc], lhsT=lhs,
                                             rhs=wt[p0:p0 + plen, g, 1],
                                             start=st, stop=sp)
                    sbE = ep.tile([128, 4, 2, 49], f32, tag="sbE")
                    nc.scalar.copy(out=sbE[:], in_=psE[:])
                    v = vp.tile([128, 4, 2, 49], bf16, tag="v")
                    nc.vector.tensor_tensor(out=v[:], in0=psO[:], in1=sbE[:], op=mmax)
                    if ei % 2 == 0:
                        pair = pp.tile([128, 2, 4, 49], bf16, tag="pair")
                    nc.vector.tensor_tensor(out=pair[:, ei % 2], in0=v[:, :, 0],
                                            in1=v[:, :, 1], op=mmax)
                    if ei % 2 == 1:
                        nc.scalar.dma_start(out=out[ei // 2], in_=pair[:])
    nc.compile()
    _NC_CACHE[key] = nc
    return nc


def _build_core_inputs(g, roi_ids, plan, flat_bf16):
    totq = plan["totq"]
    totcols = plan["chunk_col0"][NCHUNK]
    gq = np.zeros(totq, np.int16)
    wt = np.zeros((128, totcols, 196), np.float32)
    for k, n in enumerate(roi_ids):
        bid, rmin, cmin, nrows, ncols, Wfull = _roi_weights(g, n)
        pts = nrows * ncols
        rr, xx = np.divmod(np.arange(pts), ncols)
        flat_idx = (bid * (H * W) + (rmin + rr) * W + (cmin + xx)).astype(np.int16)
        qs = plan["slot_q0"][k] + np.arange(pts)
        gq[qs] = flat_idx
        Wsel = Wfull[_PERM_FLAT]                     # [196 (e,dj,pp), pts]
        wt[qs % 128, qs // 128, :] = Wsel.T
    it = np.empty((16, totq // 16), np.int16)
    qpos = np.arange(totq)
    it[qpos % 16, qpos // 16] = gq
    return {
        "bottom": flat_bf16,
        "gidx": np.tile(it, (8, 1)),
        "w": wt.reshape(128, totcols, 2, 2, 49).astype(BF16),
    }


def _prepare(bottom, rois, im_info):
    g = _grid_geometry(rois, im_info)
    pts = _roi_pts(g)
    cores = _assign_rois_to_cores(pts)
    plan = _make_plan(pts, cores)
    flat = np.ascontiguousarray(
        np.asarray(bottom, np.float32).reshape(B * H * W, C)).astype(BF16)
    in_maps = [_build_core_inputs(g, cores[c], plan, flat) for c in range(NCORES)]
    return cores, plan, in_maps


def kernel(bottom, rois, im_info):
    from concourse.bass_utils import run_bass_kernel_spmd

    cores, plan, in_maps = _prepare(bottom, rois, im_info)
    nc = _build_bass(plan)
    res = run_bass_kernel_spmd(nc, in_maps, core_ids=list(range(NCORES)))
    out = np.empty((NROI, POOL, POOL, C), np.float32)
    emit = np.array(plan["emit"])
    for c in range(NCORES):
        arr = np.asarray(res.results[c]["out"], dtype=np.float32)
        # [pair, p, r2, cc, pp] -> [pair, r2, pp, cc, p] -> [NR, 49, 512]
        vals = arr.transpose(0, 2, 4, 3, 1).reshape(NR, POOL, POOL, C)
        out[np.array(cores[c])[emit]] = vals
    return out
